# revision 1
# baseline (speedup 1.0000x reference)
"""Compensated sparse linear: out = x @ (W + delta_B)^T + b on 8 NeuronCores.

Both terms of the reference contract x against [out, in] matrices, so the
whole module is one GEMM with V = W + delta_B, plus bias.

Sharding (hardcoded for x:[4,2048,4096], W/delta_B:[4096,4096], b:[4096]):
  2 token shards x 4 out-feature shards -> 8 cores; core = r*4 + c.
  Per core: x2d shard [4096, 4096], V shard [1024, 4096] -> out [4096, 1024].

Device kernel (per core): V^T shard resident in SBUF (128KB/partition),
x^T streamed in token-blocks of 128. TensorE contracts over the partition
dim, so the host pre-tiles both operands K-major:
  xt[tb, p, kt, t] = x2d[tb*128 + t, kt*128 + p]   (16KB/partition contiguous DMA)
  vt[p, kt, n]     = V[n, kt*128 + p]
Matmuls run as fp32r (FP22 mantissa truncation, fp32 accumulate in PSUM):
1 cycle/row at moving free dim 512 vs 4 cycles for true fp32; rel err ~1.5e-4.
Bias is added by VectorE during the PSUM->SBUF copyback (replicated across
partitions host-side since it varies along the free dim).

PE executes in order, so during the 16.8MB V^T load it would starve unless
early matmul groups depend only on slices already landed:
  phase A: nh=0 groups for tb=0..PIN-1  (needs V^T half0 only, pinned xt)
  phase B: nh=1 groups for tb=0..PIN-1  (half1 streams during phase A)
  phase C: remaining t-blocks, both halves, xt streaming through the pool
"""

import numpy as np

import concourse.tile as tile
from concourse import bacc, mybir
from concourse.bass_utils import run_bass_kernel_spmd

P = 128
B, S, D_IN, D_OUT = 4, 2048, 4096, 4096
T = B * S
TR, NCOLS = 2, 4            # token shards x feature shards
T_C, N_C = T // TR, D_OUT // NCOLS
K = D_IN
TB = 128                    # tokens per t-block (psum partition dim)
NF = 512                    # matmul moving free dim (one PSUM bank, fp32)
KT = K // P
TBN = T_C // TB
NH = N_C // NF
PIN = 3                     # t-blocks pinned to cover the V^T load


def build_nc(reps=1):
    nc = bacc.Bacc("TRN2", target_bir_lowering=False, debug=False, num_devices=8)
    xt_d = nc.dram_tensor("xt", [TBN, P, KT, TB], mybir.dt.float32r, kind="ExternalInput").ap()
    vt_d = nc.dram_tensor("vt", [P, KT, N_C], mybir.dt.float32r, kind="ExternalInput").ap()
    b_d = nc.dram_tensor("bias", [P, N_C], mybir.dt.float32, kind="ExternalInput").ap()
    out_d = nc.dram_tensor("out", [T_C, N_C], mybir.dt.float32, kind="ExternalOutput").ap()

    with tile.TileContext(nc) as tc:
        with (
            tc.tile_pool(name="vt", bufs=1) as vt_pool,
            tc.tile_pool(name="bias", bufs=1) as b_pool,
            tc.tile_pool(name="xt", bufs=PIN + 1) as xt_pool,
            tc.tile_pool(name="outp", bufs=4) as out_pool,
            tc.tile_pool(name="psum", bufs=6, space="PSUM") as psum_pool,
        ):
            bias_s = b_pool.tile([P, N_C], mybir.dt.float32)

            def mm_group(xt_s, vt_s, tb, nh):
                ps = psum_pool.tile([P, NF], mybir.dt.float32)
                for kt in range(KT):
                    nc.tensor.matmul(
                        ps[:], xt_s[:, kt, :], vt_s[:, kt, :],
                        start=(kt == 0), stop=(kt == KT - 1),
                    )
                out_s = out_pool.tile([P, NF], mybir.dt.float32)
                nc.vector.tensor_add(out_s[:], ps[:], bias_s[:, nh * NF:(nh + 1) * NF])
                nc.sync.dma_start(
                    out_d[tb * TB:(tb + 1) * TB, nh * NF:(nh + 1) * NF], out_s[:]
                )

            for rep in range(reps):
                pin_tiles = {}
                # critical path first: xt0, then V^T half0
                xt_s = xt_pool.tile([P, KT, TB], mybir.dt.float32r)
                nc.sync.dma_start(xt_s[:], xt_d[0])
                pin_tiles[0] = xt_s

                vt_halves = {}
                vt_s = vt_pool.tile([P, KT, NF], mybir.dt.float32r, name="vt0")
                for kt in range(KT):
                    nc.sync.dma_start(vt_s[:, kt, :], vt_d[:, kt, 0:NF])
                vt_halves[0] = vt_s

                nc.sync.dma_start(bias_s[:], b_d[:])
                for tb in range(1, PIN):
                    xt_s = xt_pool.tile([P, KT, TB], mybir.dt.float32r)
                    nc.sync.dma_start(xt_s[:], xt_d[tb])
                    pin_tiles[tb] = xt_s

                vt_s = vt_pool.tile([P, KT, NF], mybir.dt.float32r, name="vt1")
                for kt in range(KT):
                    nc.sync.dma_start(vt_s[:, kt, :], vt_d[:, kt, NF:2 * NF])
                vt_halves[1] = vt_s

                for tb in range(PIN):          # phase A: nh0 on pinned tbs
                    mm_group(pin_tiles[tb], vt_halves[0], tb, 0)
                for tb in range(PIN):          # phase B: nh1 on pinned tbs
                    mm_group(pin_tiles[tb], vt_halves[1], tb, 1)
                for tb in range(PIN, TBN):     # phase C
                    xt_s = xt_pool.tile([P, KT, TB], mybir.dt.float32r)
                    nc.sync.dma_start(xt_s[:], xt_d[tb])
                    for nh in range(NH):
                        mm_group(xt_s, vt_halves[nh], tb, nh)
    nc.compile()
    return nc


def shard_layout():
    return [(r, c) for r in range(TR) for c in range(NCOLS)]


def prepare_in_maps(x, W, b, delta_B):
    x2d = np.asarray(x, np.float32).reshape(T, D_IN)
    V = np.asarray(W, np.float32) + np.asarray(delta_B, np.float32)
    b = np.asarray(b, np.float32)

    in_maps = []
    for r, c in shard_layout():
        xs = x2d[r * T_C:(r + 1) * T_C]
        xt = np.ascontiguousarray(xs.reshape(TBN, TB, KT, P).transpose(0, 3, 2, 1))
        Vc = V[c * N_C:(c + 1) * N_C]
        vt = np.ascontiguousarray(Vc.reshape(N_C, KT, P).transpose(2, 1, 0))
        bias = np.ascontiguousarray(np.broadcast_to(b[c * N_C:(c + 1) * N_C], (P, N_C)))
        in_maps.append({"xt": xt, "vt": vt, "bias": bias})
    return in_maps


def assemble_output(results):
    out = np.empty((T, D_OUT), np.float32)
    for i, (r, c) in enumerate(shard_layout()):
        out[r * T_C:(r + 1) * T_C, c * N_C:(c + 1) * N_C] = results[i]["out"]
    return out.reshape(B, S, D_OUT)


def kernel(x, W, b, delta_B):
    nc = build_nc()
    in_maps = prepare_in_maps(x, W, b, delta_B)
    res = run_bass_kernel_spmd(nc, in_maps, list(range(8)))
    return assemble_output(res.results)



# revision 2
# speedup vs baseline: 1.7304x; 1.7304x over previous
"""Compensated sparse linear: out = x @ (W + delta_B)^T + b on 8 NeuronCores.

Both terms contract x against [out, in] matrices, so the module is one GEMM
with V = W + delta_B, plus bias. Inputs are cast to bf16 on host (~0.3% rel
err, tolerance is 2e-2): PE runs bf16 at the same 1 row/cycle as fp32r but
FWL halves the weight-load cost, and DMA traffic halves.

Sharding (hardcoded for x:[4,2048,4096], W/delta_B:[4096,4096], b:[4096]):
  2 token shards x 4 out-feature shards -> 8 cores; core = r*4 + c.
  Per core: x2d shard [4096, 4096] tokens x K, V shard [1024, 4096].

Device kernel (per core), feature-partition PSUM:
  stationary = V^T tile [128K, 128 feat] (resident, 64KB/partition bf16)
  moving     = x^T tile [128K, 512 tok]  (streamed per t-tile, 32KB/part)
  psum       = [128 feat, 512 tok], one bank
Per (t-tile, f-window): 32 matmuls over kt, then ONE ScalarE activation
does bias-add (bias is per-partition in this orientation) + PSUM->SBUF
copy; DMA writes the transposed output od[N_C, T_C] (host untransposes).
"""

import numpy as np
import ml_dtypes

import concourse.tile as tile
from concourse import bacc, mybir
from concourse.bass_utils import run_bass_kernel_spmd

P = 128
B, S, D_IN, D_OUT = 4, 2048, 4096, 4096
T = B * S
TR, NCOLS = 2, 4            # token shards x feature shards
T_C, N_C = T // TR, D_OUT // NCOLS
K = D_IN
KT = K // P                 # 32 k-tiles
TOK = 512                   # moving free dim (one PSUM bank fp32)
TT = T_C // TOK             # 8 t-tiles
FW = N_C // P               # 8 feature windows
BF = mybir.dt.bfloat16
NPBF = ml_dtypes.bfloat16


def build_nc(reps=1):
    nc = bacc.Bacc("TRN2", target_bir_lowering=False, debug=False, num_devices=8)
    xd = nc.dram_tensor("xt", [TT, P, KT, TOK], BF, kind="ExternalInput").ap()
    vd = nc.dram_tensor("vt", [FW, P, KT, P], BF, kind="ExternalInput").ap()
    bd = nc.dram_tensor("bias", [P, FW], mybir.dt.float32, kind="ExternalInput").ap()
    od = nc.dram_tensor("out", [N_C, T_C], mybir.dt.float32, kind="ExternalOutput").ap()

    ident = mybir.ActivationFunctionType.Identity

    with tile.TileContext(nc) as tc:
        with (
            tc.tile_pool(name="v", bufs=1) as v_pool,
            tc.tile_pool(name="bias", bufs=1) as b_pool,
            tc.tile_pool(name="x", bufs=3) as x_pool,
            tc.tile_pool(name="outp", bufs=6) as out_pool,
            tc.tile_pool(name="psum", bufs=8, space="PSUM") as psum_pool,
        ):
            for rep in range(reps):
                bias_s = b_pool.tile([P, FW], mybir.dt.float32)
                v_s = v_pool.tile([P, FW, KT, P], BF, name="v")

                # critical path: first x tile, then V windows in consumption order
                x_tiles = {}
                x_s = x_pool.tile([P, KT, TOK], BF)
                nc.sync.dma_start(x_s[:], xd[0])
                x_tiles[0] = x_s
                nc.sync.dma_start(bias_s[:], bd[:])
                for fw in range(FW):
                    nc.sync.dma_start(v_s[:, fw, :, :], vd[fw])
                x_s = x_pool.tile([P, KT, TOK], BF)
                nc.sync.dma_start(x_s[:], xd[1])
                x_tiles[1] = x_s

                for tt in range(TT):
                    if tt + 2 < TT:
                        x_s = x_pool.tile([P, KT, TOK], BF)
                        nc.sync.dma_start(x_s[:], xd[tt + 2])
                        x_tiles[tt + 2] = x_s
                    xt_s = x_tiles.pop(tt)
                    for fw in range(FW):
                        ps = psum_pool.tile([P, TOK], mybir.dt.float32)
                        for kt in range(KT):
                            nc.tensor.matmul(
                                ps[:], v_s[:, fw, kt, :], xt_s[:, kt, :],
                                start=(kt == 0), stop=(kt == KT - 1),
                            )
                        o = out_pool.tile([P, TOK], mybir.dt.float32)
                        nc.scalar.activation(
                            o[:], ps[:], ident, bias=bias_s[:, fw:fw + 1], scale=1.0
                        )
                        nc.sync.dma_start(
                            od[fw * P:(fw + 1) * P, tt * TOK:(tt + 1) * TOK], o[:]
                        )
    nc.compile()
    return nc


def shard_layout():
    return [(r, c) for r in range(TR) for c in range(NCOLS)]


def prepare_in_maps(x, W, b, delta_B):
    x2d = np.asarray(x, np.float32).reshape(T, D_IN)
    V = np.asarray(W, np.float32) + np.asarray(delta_B, np.float32)
    b = np.asarray(b, np.float32)

    in_maps = []
    for r, c in shard_layout():
        xs = x2d[r * T_C:(r + 1) * T_C]
        xt = np.ascontiguousarray(
            xs.reshape(TT, TOK, KT, P).transpose(0, 3, 2, 1).astype(NPBF)
        )
        Vc = V[c * N_C:(c + 1) * N_C]
        vt = np.ascontiguousarray(
            Vc.reshape(FW, P, KT, P).transpose(0, 3, 2, 1).astype(NPBF)
        )
        bias = np.ascontiguousarray(b[c * N_C:(c + 1) * N_C].reshape(FW, P).T)
        in_maps.append({"xt": xt, "vt": vt, "bias": bias})
    return in_maps


def assemble_output(results):
    out = np.empty((T, D_OUT), np.float32)
    for i, (r, c) in enumerate(shard_layout()):
        out[r * T_C:(r + 1) * T_C, c * N_C:(c + 1) * N_C] = results[i]["out"].T
    return out.reshape(B, S, D_OUT)


def kernel(x, W, b, delta_B):
    nc = build_nc()
    in_maps = prepare_in_maps(x, W, b, delta_B)
    res = run_bass_kernel_spmd(nc, in_maps, list(range(8)))
    return assemble_output(res.results)
